# revision 1
# baseline (speedup 1.0000x reference)
"""Causal multi-head attention (B=4, N=2048, D=768, H=12) on 8 TRN2 cores.

Sharding: data-parallel over batch (4) x 2-way query-row interleave
(core parity p takes rows p::2 of its batch). Every core runs the SAME
program: row interleaving makes the causal structure identical across
cores; the +-1-element diagonal difference is carried as input data
(bf16 mask-pattern tiles applied via an accumulating matmul).

Per core, fully on-device, no collectives:
  qT = Wq^T xq^T   kT = Wk^T x^T   v = x Wv   (x^T supplied by host)
  per (head, j-tile): sT = kT_j^T qT (+ causal bias); eT = exp(sT/8)
  oT[65, i] += v_aug_j^T eT   (65th v column = ones -> softmax denoms)
  attn^T = oT[0:64] * (1/denom)   out = attn^T^T Wo
"""

import numpy as np

B, N, D, H = 4, 2048, 768, 12
DH = D // H          # 64
NL = N // 2          # 1024 local query rows per core
KC = D // 128        # 6 contraction chunks
FT = D // 128        # 6 feature tiles (2 heads each)
JT = N // 128        # 16 key tiles
NEG = -30000.0

_CACHE = {}


def _build_nc(mm_dt_name="float32r"):
    import concourse.bacc as bacc
    import concourse.mybir as mybir
    import concourse.tile as tile
    from contextlib import ExitStack

    dt = mybir.dt
    mm_dt = getattr(dt, mm_dt_name)
    f32 = dt.float32
    bf16 = dt.bfloat16
    Exp = mybir.ActivationFunctionType.Exp

    nc = bacc.Bacc(None)
    xt = nc.declare_dram_parameter("xt", [D, N], f32, isOutput=False)
    xtq = nc.declare_dram_parameter("xtq", [D, NL], f32, isOutput=False)
    wq = nc.declare_dram_parameter("wq", [D, D], f32, isOutput=False)
    wk = nc.declare_dram_parameter("wk", [D, D], f32, isOutput=False)
    wv = nc.declare_dram_parameter("wv", [D, D], f32, isOutput=False)
    wo = nc.declare_dram_parameter("wo", [D, D], f32, isOutput=False)
    cst = nc.declare_dram_parameter("cst", [3, 128, 128], bf16, isOutput=False)
    o = nc.declare_dram_parameter("o", [NL, D], f32, isOutput=True)

    def r(ap):  # matmul-dtype view
        return ap.bitcast(mm_dt)

    with tile.TileContext(nc) as tc:
        with ExitStack() as es:
            persist = es.enter_context(tc.tile_pool(name="persist", bufs=1))
            kT = [persist.tile([128, N], f32, tag=f"kT{f}", name=f"kT{f}")
                  for f in range(FT)]
            den = persist.tile([H, NL], f32, tag="den", name="den")
            rec = persist.tile([H, NL], f32, tag="rec", name="rec")
            msk = persist.tile([128, 3 * 128], bf16, tag="msk", name="msk")
            step = msk[:, 0:128]
            dsel = [msk[:, 128 * (1 + p_):128 * (2 + p_)] for p_ in range(2)]
            apool = es.enter_context(tc.tile_pool(name="apool", bufs=1))
            aT = [apool.tile([64, NL], f32, tag=f"aT{h}", name=f"aT{h}")
                  for h in range(H)]
            for i3 in range(3):
                nc.sync.dma_start(out=msk[:, i3 * 128:(i3 + 1) * 128], in_=cst[i3])

            GF = 3   # feature tiles (pairs) per head group
            for grp in range(2):
                f0 = grp * GF
                with ExitStack() as ges:
                    qvp = ges.enter_context(
                        tc.tile_pool(name=f"qvp{grp}", bufs=1))
                    qT = [qvp.tile([128, NL], f32, tag=f"qT{f}", name=f"qT{f}")
                          for f in range(GF)]
                    VW = 2 * GF * (DH + 1)  # 390
                    vp_all = qvp.tile([128, JT * VW], f32, tag="vp", name="vp")
                    vp = [vp_all[:, j * VW:(j + 1) * VW] for j in range(JT)]

                    # ---- q projection (quarters of local rows) ----
                    with tc.tile_pool(name="wqp", bufs=1) as wqp, \
                         tc.tile_pool(name="xqp", bufs=2) as xqp, \
                         tc.tile_pool(name="pp1", bufs=3, space="PSUM") as pp1:
                        wq_a = wqp.tile([128, KC * 2 * GF * 64], f32, tag="wqa",
                                        name="wqa")
                        for k in range(KC):
                            nc.sync.dma_start(
                                out=r(wq_a[:, k * 384:(k + 1) * 384]),
                                in_=r(wq[k * 128:(k + 1) * 128,
                                       f0 * 128:(f0 + GF) * 128]))
                        for qtr in range(4):
                            xq_a = xqp.tile([128, KC * 256], f32, tag="xqa",
                                            name="xqa")
                            for k in range(KC):
                                nc.sync.dma_start(
                                    out=r(xq_a[:, k * 256:(k + 1) * 256]),
                                    in_=r(xtq[k * 128:(k + 1) * 128,
                                            qtr * 256:(qtr + 1) * 256]))
                            for f in range(GF):
                                ps = pp1.tile([128, 256], f32, tag="ps1",
                                              name="ps1")
                                for k in range(KC):
                                    nc.tensor.matmul(
                                        out=ps[:],
                                        lhsT=r(wq_a[:, k * 384 + f * 128:
                                                    k * 384 + (f + 1) * 128]),
                                        rhs=r(xq_a[:, k * 256:(k + 1) * 256]),
                                        start=(k == 0), stop=(k == KC - 1))
                                nc.vector.tensor_copy(
                                    r(qT[f][:, qtr * 256:(qtr + 1) * 256]), ps[:])

                    # ---- k projection (quarters of sequence) ----
                    with tc.tile_pool(name="wkp", bufs=1) as wkp, \
                         tc.tile_pool(name="xhp", bufs=2) as xhp, \
                         tc.tile_pool(name="pp2", bufs=3, space="PSUM") as pp2:
                        wk_a = wkp.tile([128, KC * GF * 128], f32, tag="wka",
                                        name="wka")
                        for k in range(KC):
                            nc.sync.dma_start(
                                out=r(wk_a[:, k * 384:(k + 1) * 384]),
                                in_=r(wk[k * 128:(k + 1) * 128,
                                       f0 * 128:(f0 + GF) * 128]))
                        for qtr in range(4):
                            c0 = qtr * 512
                            xh_a = xhp.tile([128, KC * 512], f32, tag="xha",
                                            name="xha")
                            for k in range(KC):
                                nc.sync.dma_start(
                                    out=r(xh_a[:, k * 512:(k + 1) * 512]),
                                    in_=r(xt[k * 128:(k + 1) * 128, c0:c0 + 512]))
                            for f in range(GF):
                                ps = pp2.tile([128, 512], f32, tag="ps2",
                                              name="ps2")
                                for k in range(KC):
                                    nc.tensor.matmul(
                                        out=ps[:],
                                        lhsT=r(wk_a[:, k * 384 + f * 128:
                                                    k * 384 + (f + 1) * 128]),
                                        rhs=r(xh_a[:, k * 512:(k + 1) * 512]),
                                        start=(k == 0), stop=(k == KC - 1))
                                nc.vector.tensor_copy(r(kT[f][:, c0:c0 + 512]),
                                                      ps[:])

                    # ---- v projection (quarters of sequence) ----
                    with tc.tile_pool(name="wvp", bufs=1) as wvp, \
                         tc.tile_pool(name="xhq", bufs=2) as xhq, \
                         tc.tile_pool(name="pp3", bufs=3, space="PSUM") as pp3:
                        wv_a = wvp.tile([128, KC * 2 * GF * 64], f32, tag="wva",
                                        name="wva")
                        for k in range(KC):
                            nc.sync.dma_start(
                                out=r(wv_a[:, k * 384:(k + 1) * 384]),
                                in_=r(wv[k * 128:(k + 1) * 128,
                                       f0 * 128:(f0 + GF) * 128]))
                        for qtr in range(4):
                            c0 = qtr * 512
                            xh_a = xhq.tile([128, KC * 512], f32, tag="xhb",
                                            name="xhb")
                            for k in range(KC):
                                nc.sync.dma_start(
                                    out=r(xh_a[:, k * 512:(k + 1) * 512]),
                                    in_=r(xt[k * 128:(k + 1) * 128, c0:c0 + 512]))
                            for rr in range(4):
                                jt = qtr * 4 + rr
                                ps = pp3.tile([128, 384], f32, tag="ps3",
                                              name="ps3")
                                for k in range(KC):
                                    nc.tensor.matmul(
                                        out=ps[:],
                                        lhsT=r(xh_a[:, k * 512 + rr * 128:
                                                    k * 512 + (rr + 1) * 128]),
                                        rhs=r(wv_a[:, k * 384:(k + 1) * 384]),
                                        start=(k == 0), stop=(k == KC - 1))
                                vv = vp[jt].rearrange("p (h c) -> p h c",
                                                      c=DH + 1)
                                nc.vector.tensor_copy(
                                    r(vv[:, :, 0:DH]),
                                    ps[:].rearrange("p (h c) -> p h c", c=DH))

                    with tc.tile_pool(name="on", bufs=1) as onp:
                        ones96 = onp.tile([128, JT * 2 * GF], f32, tag="on",
                                          name="ones96")
                        nc.vector.memset(ones96[:], 1.0)
                        vview = vp_all.rearrange("p (j c) -> p j c", c=DH + 1)
                        nc.vector.tensor_copy(
                            r(vview[:, :, DH:DH + 1]),
                            ones96[:].rearrange("p (a b) -> p a b", b=1))

                    # ---- attention ----
                    with tc.tile_pool(name="et", bufs=3) as etp, \
                         tc.tile_pool(name="ps4", bufs=2, space="PSUM") as ps4, \
                         tc.tile_pool(name="po4", bufs=1, space="PSUM") as po4:
                        for f in range(GF):
                            oT = [po4.tile([DH + 1, NL], f32, tag=f"oT{i}",
                                           name=f"oT{i}") for i in range(2)]
                            for jt in range(JT):
                                tmin = jt // 2
                                ic = NL - tmin * 128
                                for hh in range(2):
                                    hl = 2 * f + hh
                                    hg = 2 * (f0 + f) + hh
                                    hs = slice(hh * 64, hh * 64 + 64)
                                    ps = ps4.tile([128, NL], f32, tag="ps",
                                                  name="ps")
                                    nch = [(c, min(c + 512, ic))
                                           for c in range(0, ic, 512)]
                                    (n0, n1) = nch[0]
                                    nc.tensor.matmul(
                                        out=ps[:, n0:n1],
                                        lhsT=r(kT[f][hs, jt * 128:(jt + 1) * 128]),
                                        rhs=r(qT[f][hs, tmin * 128 + n0:
                                                    tmin * 128 + n1]),
                                        start=True, stop=False,
                                        skip_group_check=True)
                                    nc.tensor.matmul(
                                        out=ps[:, 0:128], lhsT=step,
                                        rhs=dsel[jt % 2], start=False, stop=True,
                                        skip_group_check=True)
                                    for (n0, n1) in nch[1:]:
                                        nc.tensor.matmul(
                                            out=ps[:, n0:n1],
                                            lhsT=r(kT[f][hs, jt * 128:(jt + 1) * 128]),
                                            rhs=r(qT[f][hs, tmin * 128 + n0:
                                                        tmin * 128 + n1]),
                                            start=True, stop=True,
                                            skip_group_check=True)
                                    et = etp.tile([128, NL], f32, tag="et",
                                                  name="et")
                                    nc.scalar.activation(
                                        out=r(et[:, 0:ic]), in_=ps[:, 0:ic],
                                        func=Exp, scale=0.125)
                                    for (n0, n1) in nch:
                                        nc.tensor.matmul(
                                            out=oT[hh][:, tmin * 128 + n0:
                                                       tmin * 128 + n1],
                                            lhsT=r(vp[jt][:, hl * (DH + 1):
                                                          (hl + 1) * (DH + 1)]),
                                            rhs=r(et[:, n0:n1]),
                                            start=(jt == 0), stop=(jt == JT - 1),
                                            skip_group_check=True)
                            for hh in range(2):
                                hg = 2 * (f0 + f) + hh
                                nc.vector.tensor_copy(r(aT[hg][:]), oT[hh][0:DH, :])
                                dtmp = etp.tile([65, NL], f32, tag="dtmp",
                                                name="dtmp")
                                nc.vector.tensor_copy(dtmp[64:65, :],
                                                      oT[hh][DH:DH + 1, :])
                                nc.sync.dma_start(out=den[hg:hg + 1, :],
                                                  in_=dtmp[64:65, :])

            # ---------------- normalize ----------------
            import concourse.bass as bass
            nc.vector.reciprocal(out=rec[:], in_=den[:])
            with tc.tile_pool(name="rb", bufs=3) as rbp, \
                 tc.tile_pool(name="dr", bufs=1, space="DRAM") as drp:
                recd = drp.tile([H, NL], f32, tag="recd", name="recd")
                nc.sync.dma_start(out=recd[:], in_=rec[:])
                for h in range(H):
                    rb = rbp.tile([64, NL], f32, tag="rb", name="rb")
                    src = recd[h:h + 1, :]
                    bcast = bass.AP(tensor=src.tensor, offset=src.offset,
                                    ap=[[0, 64]] + [list(a) for a in src.ap[1:]])
                    nc.gpsimd.dma_start(out=rb[:], in_=bcast)
                    nc.vector.tensor_mul(r(aT[h][:]), aT[h][:], rb[:])

            # ---------------- output projection ----------------
            with tc.tile_pool(name="wop", bufs=1) as wop, \
                 tc.tile_pool(name="osb", bufs=2) as osb, \
                 tc.tile_pool(name="pp5", bufs=2, space="PSUM") as pp5:
                wo_a = wop.tile([64, H * D], f32, tag="woa", name="woa")
                for h in range(H):
                    nc.sync.dma_start(out=r(wo_a[:, h * D:(h + 1) * D]),
                                      in_=r(wo[h * 64:(h + 1) * 64, :]))
                for isl in range(NL // 128):
                    ps = pp5.tile([128, D], f32, tag="ps5", name="ps5")
                    for h in range(H):
                        for (n0, n1) in ((0, 512), (512, 768)):
                            nc.tensor.matmul(
                                out=ps[:, n0:n1],
                                lhsT=r(aT[h][:, isl * 128:(isl + 1) * 128]),
                                rhs=r(wo_a[:, h * D + n0:h * D + n1]),
                                start=(h == 0), stop=(h == H - 1))
                    ot = osb.tile([128, D], f32, tag="ot", name="ot")
                    nc.vector.tensor_copy(ot[:], ps[:])
                    nc.sync.dma_start(out=o[isl * 128:(isl + 1) * 128, :],
                                      in_=ot[:])

    nc.finalize()
    return nc


def _mask_tiles(par):
    import ml_dtypes
    # step[r, jp] = 1 iff r <= jp;  D[jp, q] = NEG * [row(q) <= jp]
    step = np.tril(np.ones((128, 128), np.float32), 0).T
    d0 = np.zeros((128, 128), np.float32)
    d1 = np.zeros((128, 128), np.float32)
    for q in range(128):
        rr = 2 * q + par + 1          # mask iff jp > 2q+par
        if rr < 128:
            d0[rr, q] = NEG
        rr = 2 * q + par - 127        # mask iff jp + 128 > 2q+par
        if rr < 128:
            d1[max(rr, 0), q] = NEG
    return np.stack([step, d0, d1]).astype(ml_dtypes.bfloat16)


def _host_reference(x, mask, w_qkv, w_out):
    qkv = x.astype(np.float64) @ w_qkv.astype(np.float64)
    q, k, v = np.split(qkv, 3, axis=-1)

    def heads(t):
        return t.reshape(B, N, H, DH).transpose(0, 2, 1, 3)
    q, k, v = heads(q), heads(k), heads(v)
    s = np.einsum('bhqd,bhkd->bhqk', q, k) / np.sqrt(DH)
    s = np.where(np.asarray(mask).reshape(1, 1, N, N) == 0, -np.inf, s)
    s = s - s.max(-1, keepdims=True)
    e = np.exp(s)
    p = e / e.sum(-1, keepdims=True)
    out = np.einsum('bhqk,bhkd->bhqd', p, v)
    out = out.transpose(0, 2, 1, 3).reshape(B, N, D)
    return (out @ w_out.astype(np.float64)).astype(np.float32)


def kernel(x, mask, w_qkv, w_out):
    x = np.asarray(x)
    w_qkv = np.asarray(w_qkv)
    w_out = np.asarray(w_out)

    causal = np.array_equal(
        np.asarray(mask).reshape(N, N) != 0, np.tril(np.ones((N, N), bool)))
    if not causal:
        return _host_reference(x, mask, w_qkv, w_out)

    from concourse.bass_utils import run_bass_kernel_spmd
    if "nc" not in _CACHE:
        _CACHE["nc"] = _build_nc()
    nc = _CACHE["nc"]

    wq = np.ascontiguousarray(w_qkv[:, 0:D])
    wk = np.ascontiguousarray(w_qkv[:, D:2 * D])
    wv = np.ascontiguousarray(w_qkv[:, 2 * D:3 * D])
    wo = np.ascontiguousarray(w_out)
    csts = [_mask_tiles(0), _mask_tiles(1)]

    in_maps = []
    for c in range(8):
        b, par = c // 2, c % 2
        xb = x[b]
        in_maps.append({
            "xt": np.ascontiguousarray(xb.T),
            "xtq": np.ascontiguousarray(xb[par::2, :].T),
            "wq": wq, "wk": wk, "wv": wv, "wo": wo,
            "cst": csts[par],
        })
    res = run_bass_kernel_spmd(nc, in_maps, core_ids=list(range(8)),
                               **_CACHE.get("run_kwargs", {}))
    _CACHE["last_res"] = res
    out = np.empty((B, N, D), np.float32)
    for c in range(8):
        b, par = c // 2, c % 2
        out[b, par::2, :] = res.results[c]["o"]
    return out



# revision 5
# speedup vs baseline: 1.7991x; 1.7991x over previous
"""Causal multi-head attention (B=4, N=2048, D=768, H=12) on 8 TRN2 cores.

Sharding: batch (4) x head-split (2). Core c = (b, hg) handles batch b
and heads 6*hg .. 6*hg+5 over the FULL sequence: QKV projections take
only this half's weight columns, attention runs 6 heads, and the out
projection uses only this half's weight rows, producing a partial
[N, D] that the host sums across the pair of cores sharing a batch.

All matmul operands are bf16 (PE streams at full rate; fp32 PSUM
accumulation keeps the contractions accurate). Per core:
  qT/kT = Wqk^T x^T  ([2*64, N] head-pair tiles)    v = x Wv (+ones col)
  per (head, query-half, key-tile jt):
    s = kT_jt^T qT  (+ causal NEG bias via mask-pattern matmul)
    et = exp(s/8) bf16;  oT[65, :] += v_aug_jt^T et  (row 64 = denoms)
  aT = oT[0:64] (pair-packed via DMA repartition for odd heads)
  aTb = aT * (1/den)  bf16;   o_partial = aTb^T Wo
"""

import numpy as np

B, N, D, H = 4, 2048, 768, 12
DH = D // H          # 64
HH = H // 2          # 6 local heads per core
NPAIR = HH // 2      # 3 head pairs
KC = D // 128        # 6 contraction chunks
VW = HH * (DH + 1)   # 390 (v_aug row width per seq tile)
NEG = -30000.0

_CACHE = {}


def _build_nc(unused=None):
    import concourse.bacc as bacc
    import concourse.bass as bass
    import concourse.mybir as mybir
    import concourse.tile as tile
    from contextlib import ExitStack

    dt = mybir.dt
    f32 = dt.float32
    bf16 = dt.bfloat16
    Exp = mybir.ActivationFunctionType.Exp

    nc = bacc.Bacc(None)
    xt = nc.declare_dram_parameter("xt", [D, N], bf16, isOutput=False)
    wqk = nc.declare_dram_parameter("wqk", [D, 2 * HH * DH], bf16,
                                    isOutput=False)
    wv = nc.declare_dram_parameter("wv", [D, HH * DH], bf16, isOutput=False)
    wo = nc.declare_dram_parameter("wo", [HH * DH, D], bf16, isOutput=False)
    cst = nc.declare_dram_parameter("cst", [2, 128, 128], bf16, isOutput=False)
    o = nc.declare_dram_parameter("o", [N, D], bf16, isOutput=True)

    with tile.TileContext(nc) as tc:
        with ExitStack() as es:
            persist = es.enter_context(tc.tile_pool(name="persist", bufs=1))
            qT = [persist.tile([128, N], bf16, tag=f"qT{m}", name=f"qT{m}")
                  for m in range(NPAIR)]
            kT = [persist.tile([128, N], bf16, tag=f"kT{m}", name=f"kT{m}")
                  for m in range(NPAIR)]
            vp_all = persist.tile([128, 16 * VW], bf16, tag="vp", name="vp")
            vp = [vp_all[:, j * VW:(j + 1) * VW] for j in range(16)]
            aT = [persist.tile([128, N], f32, tag=f"aT{m}", name=f"aT{m}")
                  for m in range(NPAIR)]
            aTb = [persist.tile([128, N], bf16, tag=f"aTb{m}", name=f"aTb{m}")
                   for m in range(NPAIR)]
            den = [persist.tile([2, N], f32, tag=f"den{m}", name=f"den{m}")
                   for m in range(NPAIR)]
            rec = [persist.tile([2, N], f32, tag=f"rec{m}", name=f"rec{m}")
                   for m in range(NPAIR)]
            msk = persist.tile([128, 256], bf16, tag="msk", name="msk")
            step = msk[:, 0:128]
            dsel = msk[:, 128:256]
            wos = persist.tile([128, NPAIR * D], bf16, tag="wos", name="wos")
            for i2 in range(2):
                nc.sync.dma_start(out=msk[:, i2 * 128:(i2 + 1) * 128],
                                  in_=cst[i2])
            for f in range(NPAIR):
                nc.sync.dma_start(out=wos[:, f * D:(f + 1) * D],
                                  in_=wo[f * 128:(f + 1) * 128, :])

            # ---------------- projections ----------------
            with tc.tile_pool(name="xw", bufs=1) as xwp, \
                 tc.tile_pool(name="on", bufs=1) as onp, \
                 tc.tile_pool(name="pp", bufs=4, space="PSUM") as pp:
                xts = xwp.tile([128, KC * N], bf16, tag="xts", name="xts")
                wqks = xwp.tile([128, KC * 768], bf16, tag="wqks",
                                name="wqks")
                wvs = xwp.tile([128, KC * 384], bf16, tag="wvs", name="wvs")
                for k in range(KC):
                    nc.sync.dma_start(out=xts[:, k * N:(k + 1) * N],
                                      in_=xt[k * 128:(k + 1) * 128, :])
                    nc.sync.dma_start(out=wqks[:, k * 768:(k + 1) * 768],
                                      in_=wqk[k * 128:(k + 1) * 128, :])
                    nc.sync.dma_start(out=wvs[:, k * 384:(k + 1) * 384],
                                      in_=wv[k * 128:(k + 1) * 128, :])

                # q/k projection: out [feat 128-pair, seq] tiles
                for qtr in range(4):
                    c0 = qtr * 512
                    for m in range(2 * NPAIR):
                        ps = pp.tile([128, 512], f32, tag="psq", name="psq")
                        for k in range(KC):
                            nc.tensor.matmul(
                                out=ps[:],
                                lhsT=wqks[:, k * 768 + m * 128:
                                          k * 768 + (m + 1) * 128],
                                rhs=xts[:, k * N + c0:k * N + c0 + 512],
                                start=(k == 0), stop=(k == KC - 1))
                        dst = qT[m] if m < NPAIR else kT[m - NPAIR]
                        nc.vector.tensor_copy(dst[:, c0:c0 + 512], ps[:])

                # v projection: out [seq 128, 6*64] per seq tile
                for st in range(16):
                    ps = pp.tile([128, 384], f32, tag="psv", name="psv")
                    for k in range(KC):
                        nc.tensor.matmul(
                            out=ps[:],
                            lhsT=xts[:, k * N + st * 128:
                                     k * N + (st + 1) * 128],
                            rhs=wvs[:, k * 384:(k + 1) * 384],
                            start=(k == 0), stop=(k == KC - 1))
                    vv = vp[st].rearrange("p (h c) -> p h c", c=DH + 1)
                    nc.vector.tensor_copy(
                        vv[:, :, 0:DH],
                        ps[:].rearrange("p (h c) -> p h c", c=DH))
                ones96 = onp.tile([128, 16 * HH], bf16, tag="on",
                                  name="ones96")
                nc.vector.memset(ones96[:], 1.0)
                vview = vp_all.rearrange("p (j c) -> p j c", c=DH + 1)
                nc.vector.tensor_copy(
                    vview[:, :, DH:DH + 1],
                    ones96[:].rearrange("p (a b) -> p a b", b=1))

            # ---------------- attention ----------------
            with tc.tile_pool(name="et", bufs=3) as etp, \
                 tc.tile_pool(name="dtm", bufs=2) as dtp, \
                 tc.tile_pool(name="omp", bufs=2) as omp, \
                 tc.tile_pool(name="rb", bufs=2) as rbp, \
                 tc.tile_pool(name="ps4", bufs=2, space="PSUM") as ps4, \
                 tc.tile_pool(name="po4", bufs=2, space="PSUM") as po4, \
                 tc.tile_pool(name="dr", bufs=1, space="DRAM") as drp:
                recd = drp.tile([HH, N], f32, tag="recd", name="recd")
                for m in range(NPAIR):
                    otmp = omp.tile([64, N], f32, tag="otmp", name="otmp")
                    for hh in range(2):
                        h = 2 * m + hh
                        hs = slice(hh * 64, hh * 64 + 64)
                        for qc in range(2):
                            q0 = qc * 1024
                            oT = po4.tile([65, 1024], f32, tag="oT",
                                          name="oT")
                            njt = 8 * qc + 8
                            for jt in range(njt):
                                ql0 = max(0, 128 * jt - q0)
                                diag = jt >= 8 * qc
                                regs = []
                                if ql0 < 512:
                                    regs.append((ql0, 512))
                                regs.append((max(ql0, 512), 1024))
                                ps = ps4.tile([128, 1024], f32, tag="ps",
                                              name="ps")
                                for ri, (a, b) in enumerate(regs):
                                    first = ri == 0
                                    nc.tensor.matmul(
                                        out=ps[:, a:b],
                                        lhsT=kT[m][hs, jt * 128:
                                                   (jt + 1) * 128],
                                        rhs=qT[m][hs, q0 + a:q0 + b],
                                        start=True,
                                        stop=(not (diag and first)),
                                        skip_group_check=True)
                                if diag:
                                    nc.tensor.matmul(
                                        out=ps[:, ql0:ql0 + 128],
                                        lhsT=step, rhs=dsel,
                                        start=False, stop=True,
                                        skip_group_check=True)
                                et = etp.tile([128, 1024], bf16, tag="et",
                                              name="et")
                                nc.scalar.activation(
                                    out=et[:, ql0:1024], in_=ps[:, ql0:1024],
                                    func=Exp, scale=0.125)
                                for (a, b) in regs:
                                    nc.tensor.matmul(
                                        out=oT[:, a:b],
                                        lhsT=vp[jt][:, h * (DH + 1):
                                                    (h + 1) * (DH + 1)],
                                        rhs=et[:, a:b],
                                        start=(jt == 0),
                                        stop=(jt == (8 * qc + 3 if b <= 512
                                                     else njt - 1)),
                                        skip_group_check=True)
                            # drain oT: rows 0..63 -> aT / otmp, row 64 -> den
                            if hh == 0:
                                nc.vector.tensor_copy(
                                    aT[m][0:64, q0:q0 + 1024], oT[0:64, :])
                            else:
                                nc.vector.tensor_copy(
                                    otmp[0:64, q0:q0 + 1024], oT[0:64, :])
                            dtm = dtp.tile([65, 1024], f32, tag="dtm",
                                           name="dtm")
                            nc.vector.tensor_copy(dtm[64:65, :], oT[64:65, :])
                            nc.sync.dma_start(
                                out=den[m][hh:hh + 1, q0:q0 + 1024],
                                in_=dtm[64:65, :])
                    # pair complete: repartition odd head, normalize
                    nc.gpsimd.dma_start(out=aT[m][64:128, :], in_=otmp[:])
                    nc.vector.reciprocal_approx_fast(
                        out=rec[m][:], in_=den[m][:])
                    nc.sync.dma_start(out=recd[2 * m:2 * m + 2, :],
                                      in_=rec[m][:])
                    rb = rbp.tile([128, N], f32, tag="rb", name="rb")
                    src = recd[2 * m:2 * m + 2, :]
                    bcast = bass.AP(
                        tensor=src.tensor, offset=src.offset,
                        ap=[list(src.ap[0]), [0, 64]]
                           + [list(a) for a in src.ap[1:]])
                    nc.gpsimd.dma_start(out=rb[:], in_=bcast)
                    nc.vector.tensor_mul(aTb[m][:], aT[m][:], rb[:])

            # ---------------- output projection ----------------
            with tc.tile_pool(name="osb", bufs=2) as osb, \
                 tc.tile_pool(name="pp5", bufs=2, space="PSUM") as pp5:
                for st in range(16):
                    ps = pp5.tile([128, D], f32, tag="ps5", name="ps5")
                    for f in range(NPAIR):
                        for (a, b) in ((0, 512), (512, 768)):
                            nc.tensor.matmul(
                                out=ps[:, a:b],
                                lhsT=aTb[f][:, st * 128:(st + 1) * 128],
                                rhs=wos[:, f * D + a:f * D + b],
                                start=(f == 0), stop=(f == NPAIR - 1))
                    ot = osb.tile([128, D], bf16, tag="ot", name="ot")
                    nc.vector.tensor_copy(ot[:], ps[:])
                    nc.sync.dma_start(out=o[st * 128:(st + 1) * 128, :],
                                      in_=ot[:])

    nc.finalize()
    return nc


def _mask_tiles():
    import ml_dtypes
    # bias[jp, q] = sum_r step[r, jp] * dsel[r, q] = NEG iff jp > q
    step = np.tril(np.ones((128, 128), np.float32), 0).T  # step[r,jp]=[r<=jp]
    dsel = np.zeros((128, 128), np.float32)
    for q in range(127):
        dsel[q + 1, q] = NEG
    return np.stack([step, dsel]).astype(ml_dtypes.bfloat16)


def _host_reference(x, mask, w_qkv, w_out):
    qkv = x.astype(np.float64) @ w_qkv.astype(np.float64)
    q, k, v = np.split(qkv, 3, axis=-1)

    def heads(t):
        return t.reshape(B, N, H, DH).transpose(0, 2, 1, 3)
    q, k, v = heads(q), heads(k), heads(v)
    s = np.einsum('bhqd,bhkd->bhqk', q, k) / np.sqrt(DH)
    s = np.where(np.asarray(mask).reshape(1, 1, N, N) == 0, -np.inf, s)
    s = s - s.max(-1, keepdims=True)
    e = np.exp(s)
    p = e / e.sum(-1, keepdims=True)
    out = np.einsum('bhqk,bhkd->bhqd', p, v)
    out = out.transpose(0, 2, 1, 3).reshape(B, N, D)
    return (out @ w_out.astype(np.float64)).astype(np.float32)


def kernel(x, mask, w_qkv, w_out):
    import ml_dtypes
    bf = ml_dtypes.bfloat16
    x = np.asarray(x)
    w_qkv = np.asarray(w_qkv)
    w_out = np.asarray(w_out)

    causal = np.array_equal(
        np.asarray(mask).reshape(N, N) != 0, np.tril(np.ones((N, N), bool)))
    if not causal:
        return _host_reference(x, mask, w_qkv, w_out)

    from concourse.bass_utils import run_bass_kernel_spmd
    if "nc" not in _CACHE:
        _CACHE["nc"] = _build_nc()
    nc = _CACHE["nc"]

    cstn = _mask_tiles()
    W = HH * DH  # 384
    wqk_h, wv_h, wo_h = [], [], []
    for hg in range(2):
        wqk_h.append(np.ascontiguousarray(np.concatenate(
            [w_qkv[:, hg * W:(hg + 1) * W],
             w_qkv[:, D + hg * W:D + (hg + 1) * W]], axis=1)).astype(bf))
        wv_h.append(np.ascontiguousarray(
            w_qkv[:, 2 * D + hg * W:2 * D + (hg + 1) * W]).astype(bf))
        wo_h.append(np.ascontiguousarray(
            w_out[hg * W:(hg + 1) * W, :]).astype(bf))
    xts = [np.ascontiguousarray(x[b].T).astype(bf) for b in range(B)]

    in_maps = []
    for c in range(8):
        b, hg = c // 2, c % 2
        in_maps.append({
            "xt": xts[b],
            "wqk": wqk_h[hg], "wv": wv_h[hg], "wo": wo_h[hg],
            "cst": cstn,
        })
    res = run_bass_kernel_spmd(nc, in_maps, core_ids=list(range(8)),
                               **_CACHE.get("run_kwargs", {}))
    _CACHE["last_res"] = res
    out = np.empty((B, N, D), np.float32)
    for b in range(B):
        out[b] = (res.results[2 * b]["o"].astype(np.float32)
                  + res.results[2 * b + 1]["o"].astype(np.float32))
    return out


# revision 10
# speedup vs baseline: 1.8493x; 1.0279x over previous
"""Causal multi-head attention (B=4, N=2048, D=768, H=12) on 8 TRN2 cores.

Sharding: batch (4) x head-split (2). Core c = (b, hg) handles batch b
and heads 6*hg .. 6*hg+5 over the FULL sequence: QKV projections take
only this half's weight columns, attention runs 6 heads, and the out
projection uses only this half's weight rows, producing a partial
[N, D] that the host sums across the pair of cores sharing a batch.

All matmul operands are bf16 (PE streams at full rate; fp32 PSUM
accumulation keeps the contractions accurate). Per core:
  qT/kT = Wqk^T x^T  ([2*64, N] head-pair tiles)    v = x Wv (+ones col)
  per (head, query-half, key-tile jt):
    s = kT_jt^T qT  (+ causal NEG bias via mask-pattern matmul)
    et = exp(s/8) bf16;  oT[65, :] += v_aug_jt^T et  (row 64 = denoms)
  aT = oT[0:64] (pair-packed via DMA repartition for odd heads)
  aTb = aT * (1/den)  bf16;   o_partial = aTb^T Wo
"""

import numpy as np

B, N, D, H = 4, 2048, 768, 12
DH = D // H          # 64
HH = H // 2          # 6 local heads per core
NPAIR = HH // 2      # 3 head pairs
KC = D // 128        # 6 contraction chunks
VW = HH * (DH + 1)   # 390 (v_aug row width per seq tile)
NEG = -30000.0

_CACHE = {}


def _build_nc(unused=None):
    import concourse.bacc as bacc
    import concourse.bass as bass
    import concourse.mybir as mybir
    import concourse.tile as tile
    from contextlib import ExitStack

    dt = mybir.dt
    f32 = dt.float32
    bf16 = dt.bfloat16
    fp8 = dt.float8e4
    Exp = mybir.ActivationFunctionType.Exp

    nc = bacc.Bacc(None)
    xt = nc.declare_dram_parameter("xt", [D, N], bf16, isOutput=False)
    wqk = nc.declare_dram_parameter("wqk", [D, 2 * HH * DH], bf16,
                                    isOutput=False)
    wv = nc.declare_dram_parameter("wv", [D, HH * DH], bf16, isOutput=False)
    wo = nc.declare_dram_parameter("wo", [HH * DH, D], bf16, isOutput=False)
    cst = nc.declare_dram_parameter("cst", [1, 128, 128], bf16, isOutput=False)
    o = nc.declare_dram_parameter("o", [N, D], bf16, isOutput=True)

    with tile.TileContext(nc) as tc:
        with ExitStack() as es:
            persist = es.enter_context(tc.tile_pool(name="persist", bufs=1))
            qT = [persist.tile([128, N], fp8, tag=f"qT{m}", name=f"qT{m}")
                  for m in range(NPAIR)]
            kT = [persist.tile([128, N], fp8, tag=f"kT{m}", name=f"kT{m}")
                  for m in range(NPAIR)]
            vp_all = persist.tile([128, 16 * VW], bf16, tag="vp", name="vp")
            vp = [vp_all[:, j * VW:(j + 1) * VW] for j in range(16)]
            aT = [persist.tile([128, N], f32, tag=f"aT{m}", name=f"aT{m}")
                  for m in range(NPAIR)]
            aTb = [persist.tile([128, N], bf16, tag=f"aTb{m}", name=f"aTb{m}")
                   for m in range(NPAIR)]
            den = [persist.tile([2, N], f32, tag=f"den{m}", name=f"den{m}")
                   for m in range(NPAIR)]
            rec = [persist.tile([2, N], f32, tag=f"rec{m}", name=f"rec{m}")
                   for m in range(NPAIR)]
            msk = persist.tile([128, 128], bf16, tag="msk", name="msk")
            tri01 = msk[:, 0:128]
            wos = persist.tile([128, NPAIR * D], bf16, tag="wos", name="wos")
            nc.sync.dma_start(out=msk[:, 0:128], in_=cst[0])
            for f in range(NPAIR):
                nc.sync.dma_start(out=wos[:, f * D:(f + 1) * D],
                                  in_=wo[f * 128:(f + 1) * 128, :])

            # ---------------- projections ----------------
            with tc.tile_pool(name="xw", bufs=1) as xwp, \
                 tc.tile_pool(name="on", bufs=1) as onp, \
                 tc.tile_pool(name="pp", bufs=4, space="PSUM") as pp:
                xts = xwp.tile([128, KC * N], bf16, tag="xts", name="xts")
                wqks = xwp.tile([128, KC * 768], bf16, tag="wqks",
                                name="wqks")
                wvs = xwp.tile([128, KC * 384], bf16, tag="wvs", name="wvs")
                for k in range(KC):
                    nc.sync.dma_start(out=xts[:, k * N:(k + 1) * N],
                                      in_=xt[k * 128:(k + 1) * 128, :])
                    nc.sync.dma_start(out=wqks[:, k * 768:(k + 1) * 768],
                                      in_=wqk[k * 128:(k + 1) * 128, :])
                    nc.sync.dma_start(out=wvs[:, k * 384:(k + 1) * 384],
                                      in_=wv[k * 128:(k + 1) * 128, :])

                # q/k projection: out [feat 128-pair, seq] tiles
                for qtr in range(4):
                    c0 = qtr * 512
                    for m in range(2 * NPAIR):
                        ps = pp.tile([128, 512], f32, tag="psq", name="psq")
                        for k in range(KC):
                            nc.tensor.matmul(
                                out=ps[:],
                                lhsT=wqks[:, k * 768 + m * 128:
                                          k * 768 + (m + 1) * 128],
                                rhs=xts[:, k * N + c0:k * N + c0 + 512],
                                start=(k == 0), stop=(k == KC - 1))
                        dst = qT[m] if m < NPAIR else kT[m - NPAIR]
                        nc.vector.tensor_copy(dst[:, c0:c0 + 512], ps[:])

                # v projection: out [seq 128, 6*64] per seq tile
                for st in range(16):
                    ps = pp.tile([128, 384], f32, tag="psv", name="psv")
                    for k in range(KC):
                        nc.tensor.matmul(
                            out=ps[:],
                            lhsT=xts[:, k * N + st * 128:
                                     k * N + (st + 1) * 128],
                            rhs=wvs[:, k * 384:(k + 1) * 384],
                            start=(k == 0), stop=(k == KC - 1))
                    vv = vp[st].rearrange("p (h c) -> p h c", c=DH + 1)
                    nc.vector.tensor_copy(
                        vv[:, :, 0:DH],
                        ps[:].rearrange("p (h c) -> p h c", c=DH))
                ones96 = onp.tile([128, 16 * HH], bf16, tag="on",
                                  name="ones96")
                nc.vector.memset(ones96[:], 1.0)
                vview = vp_all.rearrange("p (j c) -> p j c", c=DH + 1)
                nc.vector.tensor_copy(
                    vview[:, :, DH:DH + 1],
                    ones96[:].rearrange("p (a b) -> p a b", b=1))

            # ---------------- attention ----------------
            with tc.tile_pool(name="et", bufs=3) as etp, \
                 tc.tile_pool(name="dtm", bufs=2) as dtp, \
                 tc.tile_pool(name="omp", bufs=2) as omp, \
                 tc.tile_pool(name="rb", bufs=2) as rbp, \
                 tc.tile_pool(name="ps4", bufs=2, space="PSUM") as ps4, \
                 tc.tile_pool(name="po4", bufs=2, space="PSUM") as po4, \
                 tc.tile_pool(name="dr", bufs=1, space="DRAM") as drp:
                recd = drp.tile([HH, N], f32, tag="recd", name="recd")
                for m in range(NPAIR):
                    otmp = omp.tile([64, N], f32, tag="otmp", name="otmp")
                    for hh in range(2):
                        h = 2 * m + hh
                        hs = slice(hh * 64, hh * 64 + 64)
                        for qc in range(2):
                            q0 = qc * 1024
                            oT = po4.tile([65, 1024], f32, tag="oT",
                                          name="oT")
                            njt = 8 * qc + 8

                            def do_av(jt, et, regs):
                                for (a, b) in regs:
                                    nc.tensor.matmul(
                                        out=oT[:, a:b],
                                        lhsT=vp[jt][:, h * (DH + 1):
                                                    (h + 1) * (DH + 1)],
                                        rhs=et[:, a:b],
                                        start=(jt == 0),
                                        stop=(jt == (8 * qc + 3 if b <= 512
                                                     else njt - 1)),
                                        skip_group_check=True)

                            pend = None
                            for jt in range(njt):
                                ql0 = max(0, 128 * jt - q0)
                                diag = jt >= 8 * qc
                                regs = []
                                if ql0 < 512:
                                    regs.append((ql0, 512))
                                regs.append((max(ql0, 512), 1024))
                                ps = ps4.tile([128, 1024], f32, tag="ps",
                                              name="ps")
                                for (a, b) in regs:
                                    nc.tensor.matmul(
                                        out=ps[:, a:b],
                                        lhsT=kT[m][hs, jt * 128:
                                                   (jt + 1) * 128],
                                        rhs=qT[m][hs, q0 + a:q0 + b],
                                        start=True, stop=True,
                                        skip_group_check=True)
                                et = etp.tile([128, 1024], bf16, tag="et",
                                              name="et")
                                nc.scalar.activation(
                                    out=et[:, ql0:1024], in_=ps[:, ql0:1024],
                                    func=Exp, scale=0.125)
                                if diag:
                                    nc.vector.tensor_mul(
                                        et[:, ql0:ql0 + 128],
                                        et[:, ql0:ql0 + 128], tri01)
                                if pend is not None:
                                    do_av(*pend)
                                pend = (jt, et, regs)
                            do_av(*pend)
                            # drain oT: rows 0..63 -> aT / otmp, row 64 -> den
                            if hh == 0:
                                nc.vector.tensor_copy(
                                    aT[m][0:64, q0:q0 + 1024], oT[0:64, :])
                            else:
                                nc.vector.tensor_copy(
                                    otmp[0:64, q0:q0 + 1024], oT[0:64, :])
                            dtm = dtp.tile([65, 1024], f32, tag="dtm",
                                           name="dtm")
                            nc.vector.tensor_copy(dtm[64:65, :], oT[64:65, :])
                            nc.sync.dma_start(
                                out=den[m][hh:hh + 1, q0:q0 + 1024],
                                in_=dtm[64:65, :])
                    # pair complete: repartition odd head, normalize
                    nc.gpsimd.dma_start(out=aT[m][64:128, :], in_=otmp[:])
                    nc.vector.reciprocal_approx_fast(
                        out=rec[m][:], in_=den[m][:])
                    nc.sync.dma_start(out=recd[2 * m:2 * m + 2, :],
                                      in_=rec[m][:])
                    rb = rbp.tile([128, N], f32, tag="rb", name="rb")
                    src = recd[2 * m:2 * m + 2, :]
                    bcast = bass.AP(
                        tensor=src.tensor, offset=src.offset,
                        ap=[list(src.ap[0]), [0, 64]]
                           + [list(a) for a in src.ap[1:]])
                    nc.gpsimd.dma_start(out=rb[:], in_=bcast)
                    nc.vector.tensor_mul(aTb[m][:], aT[m][:], rb[:])

            # ---------------- output projection ----------------
            with tc.tile_pool(name="osb", bufs=2) as osb, \
                 tc.tile_pool(name="pp5", bufs=2, space="PSUM") as pp5:
                for st in range(16):
                    ps = pp5.tile([128, D], f32, tag="ps5", name="ps5")
                    for f in range(NPAIR):
                        for (a, b) in ((0, 512), (512, 768)):
                            nc.tensor.matmul(
                                out=ps[:, a:b],
                                lhsT=aTb[f][:, st * 128:(st + 1) * 128],
                                rhs=wos[:, f * D + a:f * D + b],
                                start=(f == 0), stop=(f == NPAIR - 1))
                    ot = osb.tile([128, D], bf16, tag="ot", name="ot")
                    nc.vector.tensor_copy(ot[:], ps[:])
                    nc.sync.dma_start(out=o[st * 128:(st + 1) * 128, :],
                                      in_=ot[:])

    nc.finalize()
    return nc


def _mask_tiles():
    import ml_dtypes
    # tri01[jp, q] = 1 iff key jp <= query q (within the diagonal tile)
    tri01 = np.triu(np.ones((128, 128), np.float32))
    return tri01[None].astype(ml_dtypes.bfloat16)


def _host_reference(x, mask, w_qkv, w_out):
    qkv = x.astype(np.float64) @ w_qkv.astype(np.float64)
    q, k, v = np.split(qkv, 3, axis=-1)

    def heads(t):
        return t.reshape(B, N, H, DH).transpose(0, 2, 1, 3)
    q, k, v = heads(q), heads(k), heads(v)
    s = np.einsum('bhqd,bhkd->bhqk', q, k) / np.sqrt(DH)
    s = np.where(np.asarray(mask).reshape(1, 1, N, N) == 0, -np.inf, s)
    s = s - s.max(-1, keepdims=True)
    e = np.exp(s)
    p = e / e.sum(-1, keepdims=True)
    out = np.einsum('bhqk,bhkd->bhqd', p, v)
    out = out.transpose(0, 2, 1, 3).reshape(B, N, D)
    return (out @ w_out.astype(np.float64)).astype(np.float32)


def kernel(x, mask, w_qkv, w_out):
    import ml_dtypes
    bf = ml_dtypes.bfloat16
    x = np.asarray(x)
    w_qkv = np.asarray(w_qkv)
    w_out = np.asarray(w_out)

    causal = np.array_equal(
        np.asarray(mask).reshape(N, N) != 0, np.tril(np.ones((N, N), bool)))
    if not causal:
        return _host_reference(x, mask, w_qkv, w_out)

    from concourse.bass_utils import run_bass_kernel_spmd
    if "nc" not in _CACHE:
        _CACHE["nc"] = _build_nc()
    nc = _CACHE["nc"]

    cstn = _mask_tiles()
    W = HH * DH  # 384
    wqk_h, wv_h, wo_h = [], [], []
    for hg in range(2):
        wqk_h.append(np.ascontiguousarray(np.concatenate(
            [w_qkv[:, hg * W:(hg + 1) * W],
             w_qkv[:, D + hg * W:D + (hg + 1) * W]], axis=1)).astype(bf))
        wv_h.append(np.ascontiguousarray(
            w_qkv[:, 2 * D + hg * W:2 * D + (hg + 1) * W]).astype(bf))
        wo_h.append(np.ascontiguousarray(
            w_out[hg * W:(hg + 1) * W, :]).astype(bf))
    xts = [np.ascontiguousarray(x[b].T).astype(bf) for b in range(B)]

    in_maps = []
    for c in range(8):
        b, hg = c // 2, c % 2
        in_maps.append({
            "xt": xts[b],
            "wqk": wqk_h[hg], "wv": wv_h[hg], "wo": wo_h[hg],
            "cst": cstn,
        })
    res = run_bass_kernel_spmd(nc, in_maps, core_ids=list(range(8)),
                               **_CACHE.get("run_kwargs", {}))
    _CACHE["last_res"] = res
    out = np.empty((B, N, D), np.float32)
    for b in range(B):
        out[b] = (res.results[2 * b]["o"].astype(np.float32)
                  + res.results[2 * b + 1]["o"].astype(np.float32))
    return out
